# revision 48
# baseline (speedup 1.0000x reference)
"""Causal single-head attention (B=4, T=4096, C=2048, H=128) on 8 TRN2 cores.

Sharding: data-parallel over batch (2 cores per batch element). Within a
batch, core half h owns query tiles qt with qt mod 4 in {2h, 2h+1} — both
cores get an identical multiset of causal key-block counts, so one SPMD
program is balanced. Each core also projects k/v/q only for its own 2048
columns; (k^T | v^T) halves are exchanged with a pair-wise AllGather per
TWO 512-column groups (4 collectives total), halving both the x DMA
traffic and the k/v projection FLOPs.

Per-core device program (fp16 operands, f32 PSUM accumulation), pipelined
per column group g: project k^T / v^T / q^T of my 256 columns from slab
g, AllGather (k^T | v^T) of groups (2i, 2i+1) with the pair partner, and
interleave one attention q-group (4 query tiles, 512 q columns) per two
projection groups, in the transposed S^T layout per 256-key chunk pair:
  S^T pair [s=2x128, q=512] (PE) -> exp (ACT) -> x 0/1 causal mask (DVE,
  diagonal chunks only) -> row-sums via ones-matmul + out^T AV
  accumulation (PE) -> out^T * (1/sums) (DVE approx-recip + mul) ->
  +bv, cast (ACT, per-partition bias in the out^T layout) -> DMA out^T.
The output is returned transposed ([H, TQ] per core); the host undoes it.
"""

import ml_dtypes
import numpy as np

import concourse.bacc as bacc
import concourse.mybir as mybir
import concourse.tile as tile
from concourse.bass_utils import run_bass_kernel_spmd

B, T, C, H = 4, 4096, 2048, 128
P = 128          # partitions / head dim / q tile
KB = 512         # free-dim tile (one f32 PSUM bank)
HB = 256         # per-core half of a column group
NQT = 16         # query tiles per core
TQ = NQT * P     # query rows per core
NCC = C // P     # contraction chunks (16)
NG = T // KB     # 512-wide column groups (8)
NM = 4           # attention q-groups per core (4 tiles each)
NMSK = 8 * NM    # 8 masked 128-key chunks per q-group

F16 = np.float16
F8 = ml_dtypes.float8_e4m3fn
_NC_CACHE = {}
REPLICA_GROUPS = [[0, 1], [2, 3], [4, 5], [6, 7]]


def _qtiles_for(half):
    # global query-tile ids, j-th tile of this core; kb counts [1,1,2,2,...,8,8]
    return [4 * (j // 2) + 2 * half + (j % 2) for j in range(NQT)]


def build_nc():
    dt = mybir.dt
    nc = bacc.Bacc("TRN2", target_bir_lowering=False, debug=False, num_devices=8)

    xP = nc.dram_tensor("xP", [NG, P, NCC, HB], dt.float16, kind="ExternalInput").ap()
    wk8 = nc.dram_tensor("wk8", [P, NCC, H], dt.float8e4, kind="ExternalInput").ap()
    wq8 = nc.dram_tensor("wq8", [P, NCC, H], dt.float8e4, kind="ExternalInput").ap()
    wv = nc.dram_tensor("wv", [P, NCC, H], dt.float16, kind="ExternalInput").ap()
    bk = nc.dram_tensor("bk", [P, 1], dt.float32, kind="ExternalInput").ap()
    bq = nc.dram_tensor("bq", [P, 1], dt.float32, kind="ExternalInput").ap()
    bv = nc.dram_tensor("bv", [P, 1], dt.float32, kind="ExternalInput").ap()
    consts = nc.dram_tensor("consts", [P, 2, P], dt.float16, kind="ExternalInput").ap()
    # causal-mask index tile: I[p, h2, q~] = q_global_local - 128*h2 - p;
    # keep key (2*pi + h2, p) for q~ of group m iff I >= 128*(2*pi - 8*m)
    imask = nc.dram_tensor("imask", [P, 2, KB], dt.float16, kind="ExternalInput").ap()
    outT = nc.dram_tensor("outT", [P, NM, KB], dt.float32, kind="ExternalOutput").ap()

    Exp = mybir.ActivationFunctionType.Exp
    Ident = mybir.ActivationFunctionType.Identity

    with tile.TileContext(nc) as tc:
        with (
            tc.tile_pool(name="wpool", bufs=1) as wpool,
            tc.tile_pool(name="cpool", bufs=1) as cpool,
            tc.tile_pool(name="persist", bufs=1) as persist,
            tc.tile_pool(name="mpool", bufs=1) as mpool,
            tc.tile_pool(name="xpool", bufs=5) as xpool,
            tc.tile_pool(name="vtpool", bufs=2) as vtpool,
            tc.tile_pool(name="kvpool", bufs=2) as kvpool,
            tc.tile_pool(name="dram", bufs=4, space="DRAM") as dram,
            tc.tile_pool(name="ppool", bufs=2, space="PSUM") as ppool,
            tc.tile_pool(name="spool", bufs=2, space="PSUM") as spool,
            tc.tile_pool(name="sumpool", bufs=1, space="PSUM") as sumpool,
            tc.tile_pool(name="otpool", bufs=1, space="PSUM") as otpool,
            tc.tile_pool(name="weipool", bufs=5) as weipool,
            tc.tile_pool(name="statpool", bufs=2) as statpool,
            tc.tile_pool(name="osbpool", bufs=2) as osbpool,
            tc.tile_pool(name="opool", bufs=2) as opool,
        ):
            # ---- constants (tiny ones first so slab 0 lands early) ----
            bk_t = cpool.tile([P, 1], dt.float32, tag="bk")
            bq_t = cpool.tile([P, 1], dt.float32, tag="bq")
            bv_t = cpool.tile([P, 1], dt.float32, tag="bv")
            idon = cpool.tile([P, 2, P], dt.float16, tag="idon")
            nc.sync.dma_start(bk_t[:], bk)
            nc.sync.dma_start(bq_t[:], bq)
            nc.sync.dma_start(bv_t[:], bv)
            nc.sync.dma_start(idon[:], consts)
            wk_t = wpool.tile([P, NCC, H], dt.float8e4, tag="wk")
            wq_t = wpool.tile([P, NCC, H], dt.float8e4, tag="wq")
            wv_t = wpool.tile([P, NCC, H], dt.float16, tag="wv")
            nc.sync.dma_start(wk_t[:], wk8)
            imask_t = mpool.tile([P, 2, KB], dt.float16, tag="imask")
            nc.sync.dma_start(imask_t[:], imask)

            kT = persist.tile([P, T], dt.float16, tag="kT")
            qT = persist.tile([P, TQ], dt.float16, tag="qT")
            vS = persist.tile([P, T // P, H], dt.float16, tag="vS")

            DR = mybir.MatmulPerfMode.DoubleRow
            fetched = {}

            def fetch(g):
                # DMA slab g and cast it to fp8 (issued ahead of use so the
                # cast sits early in the DVE queue)
                if g >= NG or g in fetched:
                    return
                xs = xpool.tile([P, NCC, HB], dt.float16, tag="xs", name=f"xs{g}")
                x8 = xpool.tile([P, NCC, HB], dt.float8e4, tag="x8", name=f"x8{g}")
                nc.sync.dma_start(xs[:], xP[g])
                nc.vector.tensor_copy(x8[:], xs[:])
                fetched[g] = (xs, x8)

            def project(g, kv2):
                xs, x8 = fetched.pop(g)
                fetch(g + 3)
                if g == 3:
                    fetch(7)
                gg = g % 2
                # k^T of my half (fp8 DoubleRow, weights pre-scaled by 2^5)
                pk = ppool.tile([P, HB], dt.float32, tag="proj")
                for j in range(NCC // 2):
                    nc.tensor.matmul(
                        pk[:], lhsT=wk_t[:, 2 * j : 2 * j + 2, :],
                        rhs=x8[:, 2 * j : 2 * j + 2, :],
                        start=(j == 0), stop=(j == NCC // 2 - 1),
                        perf_mode=DR,
                    )
                nc.scalar.activation(
                    kv2[:, gg, 0, :], pk[:], Ident, bias=bk_t[:], scale=2.0 ** -5
                )
                # q^T for my two tiles (fp8 DoubleRow, weights pre-scaled by 2^10)
                pq = ppool.tile([P, HB], dt.float32, tag="proj")
                for j in range(NCC // 2):
                    nc.tensor.matmul(
                        pq[:], lhsT=wq_t[:, 2 * j : 2 * j + 2, :],
                        rhs=x8[:, 2 * j : 2 * j + 2, :],
                        start=(j == 0), stop=(j == NCC // 2 - 1),
                        perf_mode=DR,
                    )
                nc.scalar.activation(
                    qT[:, HB * g : HB * (g + 1)], pq[:], Ident, bias=bq_t[:],
                    scale=2.0 ** -10,
                )
                # v^T of my half (fp16)
                pv = ppool.tile([P, HB], dt.float32, tag="proj")
                for cc in range(NCC):
                    nc.tensor.matmul(
                        pv[:], lhsT=wv_t[:, cc, :], rhs=xs[:, cc, :],
                        start=(cc == 0), stop=(cc == NCC - 1),
                    )
                vt = vtpool.tile([P, HB], dt.float16, tag="vt")
                nc.scalar.copy(vt[:], pv[:])
                for s4 in range(2):
                    tp = ppool.tile([P, P], dt.float16, tag="proj")
                    nc.tensor.transpose(
                        tp[:], vt[:, P * s4 : P * (s4 + 1)], idon[:, 0, :]
                    )
                    nc.vector.tensor_copy(
                        kv2[:, gg, 1, P * s4 : P * (s4 + 1)], tp[:]
                    )

            def exchange(i):
                # AllGather (k^T | v^T) of groups 2i, 2i+1 with the pair partner.
                # cin staging on the scalar ring: the trigger lands right after
                # the k/q activations that produce kv, so it neither waits long
                # nor blocks the x-slab stream on the sync ring.
                cin = dram.tile([P, 2, 2, HB], dt.float16, tag="cin")
                cout = dram.tile([2, P, 2, 2, HB], dt.float16, tag="cout")
                nc.scalar.dma_start(cin[:], kvx[i % 2][:])
                nc.gpsimd.collective_compute(
                    "AllGather",
                    mybir.AluOpType.bypass,
                    replica_groups=REPLICA_GROUPS,
                    ins=[cin.opt()],
                    outs=[cout.opt()],
                )
                return cout

            def unpack(i, cout):
                # cout[r, :, gg] = (k^T | v^T) of group 2i+gg from core-half r
                for gg in range(2):
                    g = 2 * i + gg
                    nc.sync.dma_start(
                        kT[:, KB * g : KB * (g + 1)].rearrange(
                            "p (r h) -> p r h", r=2
                        ),
                        cout[:, :, gg, 0, :].rearrange("r p h -> p r h"),
                    )
                    nc.sync.dma_start(
                        vS[:, 4 * g : 4 * (g + 1), :].rearrange(
                            "p (r s) h -> p r s h", r=2
                        ),
                        cout[:, :, gg, 1, :].rearrange(
                            "r p (s h) -> p r s h", s=2
                        ),
                    )

            def attention(m):
                nch = 8 * m + 8     # 128-wide key chunks for this group
                npr = nch // 2
                sums = sumpool.tile([P, KB], dt.float32, tag="sums")
                otp = otpool.tile([P, KB], dt.float32, tag="otp")
                qg = qT[:, KB * m : KB * (m + 1)]
                wei_tiles = []

                def ones_av(p):
                    w = wei_tiles[p]
                    for h2 in range(2):
                        c = 2 * p + h2
                        nc.tensor.matmul(
                            sums[:], lhsT=idon[:, 1, :], rhs=w[:, h2, :],
                            start=(c == 0), stop=(c == nch - 1),
                        )
                        nc.tensor.matmul(
                            otp[:], lhsT=vS[:, c, :], rhs=w[:, h2, :],
                            start=(c == 0), stop=(c == nch - 1),
                        )

                for p in range(npr):
                    st = spool.tile([P, 2, KB], dt.float32, tag="st")
                    for h2 in range(2):
                        nc.tensor.matmul(
                            st[:, h2, :],
                            lhsT=kT[:, P * (2 * p + h2) : P * (2 * p + h2 + 1)],
                            rhs=qg, start=True, stop=True,
                        )
                    wei = weipool.tile([P, 2, KB], dt.float16, tag="wei")
                    nc.scalar.activation(wei[:], st[:], Exp)
                    if p >= npr - 4:
                        # causal mask: keep iff imask >= 128*(2*p - 8*m)
                        nc.vector.scalar_tensor_tensor(
                            wei[:], imask_t[:], float(128 * (2 * p - 8 * m)),
                            wei[:],
                            op0=mybir.AluOpType.is_ge,
                            op1=mybir.AluOpType.mult,
                        )
                    wei_tiles.append(wei)
                    if p > 0:
                        ones_av(p - 1)
                ones_av(npr - 1)
                rec = statpool.tile([P, KB], dt.float32, tag="rec")
                nc.vector.reciprocal_approx_fast(rec[:], sums[:])
                osb = osbpool.tile([P, KB], dt.float16, tag="osb")
                nc.vector.tensor_mul(osb[:], otp[:], rec[:])
                oT = opool.tile([P, KB], dt.float32, tag="oT")
                nc.scalar.activation(oT[:], osb[:], Ident, bias=bv_t[:])
                # scalar ring: fires right after the producing activation,
                # keeps the sync ring free for the x-slab stream
                nc.scalar.dma_start(outT[:, m, :], oT[:])

            # warm-up collective: absorbs CC-core boot + barrier skew while
            # the x/weight DMAs stream in
            cin_w = dram.tile([P, 2], dt.float16, tag="cinw")
            cout_w = dram.tile([2, P, 2], dt.float16, tag="coutw")
            nc.gpsimd.dma_start(cin_w[:], idon[:, 0, 0:2])
            nc.gpsimd.collective_compute(
                "AllGather",
                mybir.AluOpType.bypass,
                replica_groups=REPLICA_GROUPS,
                ins=[cin_w.opt()],
                outs=[cout_w.opt()],
            )

            # pipeline: projections feed attention groups as kT/vS fill in
            kvx = [
                kvpool.tile([P, 2, 2, HB], dt.float16, tag="kvx", name="kvx0"),
                kvpool.tile([P, 2, 2, HB], dt.float16, tag="kvx", name="kvx1"),
            ]
            fetch(0)
            nc.sync.dma_start(wq_t[:], wq8)
            nc.sync.dma_start(wv_t[:], wv)
            fetch(1)
            fetch(2)
            project(0, kvx[0])
            project(1, kvx[0])
            co0 = exchange(0)
            project(2, kvx[1])
            project(3, kvx[1])
            co1 = exchange(1)
            unpack(0, co0)
            attention(0)
            project(4, kvx[0])
            project(5, kvx[0])
            co2 = exchange(2)
            unpack(1, co1)
            attention(1)
            project(6, kvx[1])
            project(7, kvx[1])
            co3 = exchange(3)
            unpack(2, co2)
            unpack(3, co3)
            attention(2)
            attention(3)

    nc.compile()
    return nc


def _host_prep(x, Wk, bk, Wq, bq, Wv, bv):
    scale = float(C) ** -0.5

    def tile_w(w):
        return np.ascontiguousarray(
            w.reshape(NCC, P, H).transpose(1, 0, 2)
        )

    wk8c = tile_w((np.asarray(Wk, np.float32) * 2.0 ** 5).astype(F8))
    wq8c = tile_w((np.asarray(Wq, np.float32) * scale * 2.0 ** 10).astype(F8))
    wv16 = tile_w(np.asarray(Wv, np.float32).astype(F16))
    bk_c = np.asarray(bk, np.float32).reshape(P, 1)
    bq_c = (np.asarray(bq, np.float32) * scale).reshape(P, 1)
    bv_c = np.asarray(bv, np.float32).reshape(P, 1)
    consts = np.ascontiguousarray(
        np.stack([np.eye(P, dtype=F16), np.ones((P, P), F16)]).transpose(1, 0, 2)
    )

    # causal-mask index tile per half (m-independent; see kernel docstring)
    imask_by_half = []
    for half in (0, 1):
        I = np.empty((P, 2, KB), np.float32)
        for r in range(4):
            qloc = P * (2 * half + 4 * (r // 2) + (r % 2)) + np.arange(KB // 4)
            for h2 in range(2):
                I[:, h2, (KB // 4) * r : (KB // 4) * (r + 1)] = (
                    qloc[None, :] - P * h2 - np.arange(P)[:, None]
                )
        imask_by_half.append(I.astype(F16))

    in_maps = []
    for core in range(8):
        b_idx, half = core // 2, core % 2
        xTb = np.ascontiguousarray(np.asarray(x[b_idx], np.float32).T)
        xPc = np.empty((NG, P, NCC, HB), F16)
        for g in range(NG):
            grp = xTb[:, KB * g + HB * half : KB * g + HB * (half + 1)]
            xPc[g] = grp.reshape(NCC, P, HB).transpose(1, 0, 2).astype(F16)
        in_maps.append({
            "xP": xPc,
            "wk8": wk8c, "wq8": wq8c, "wv": wv16,
            "bk": bk_c, "bq": bq_c, "bv": bv_c,
            "consts": consts, "imask": imask_by_half[half],
        })
    return in_maps


def kernel(x, Wk, bk, Wq, bq, Wv, bv):
    if "nc" not in _NC_CACHE:
        _NC_CACHE["nc"] = build_nc()
    nc = _NC_CACHE["nc"]
    in_maps = _host_prep(x, Wk, bk, Wq, bq, Wv, bv)
    res = run_bass_kernel_spmd(nc, in_maps, list(range(8))).results
    out = np.empty((B, T, H), np.float32)
    for core in range(8):
        b_idx, half = core // 2, core % 2
        oT = res[core]["outT"]  # [P(H), NM, KB]
        qts = _qtiles_for(half)
        for j, qt in enumerate(qts):
            m, r = j // 4, j % 4
            out[b_idx, qt * P : (qt + 1) * P, :] = oT[:, m, r * P : (r + 1) * P].T
    return out


# revision 50
# speedup vs baseline: 1.0704x; 1.0704x over previous
"""Causal single-head attention (B=4, T=4096, C=2048, H=128) on 8 TRN2 cores.

Sharding: data-parallel over batch (2 cores per batch element). Within a
batch, core half h owns query tiles qt with qt mod 4 in {2h, 2h+1} — both
cores get an identical multiset of causal key-block counts, so one SPMD
program is balanced. Each core also projects k/v/q only for its own 2048
columns; (k^T | v^T) halves are exchanged with a pair-wise AllGather per
TWO 512-column groups (4 collectives total), halving both the x DMA
traffic and the k/v projection FLOPs.

Per-core device program (fp16 operands, f32 PSUM accumulation), pipelined
per column group g: project k^T / v^T / q^T of my 256 columns from slab
g, AllGather (k^T | v^T) of groups (2i, 2i+1) with the pair partner, and
interleave one attention q-group (4 query tiles, 512 q columns) per two
projection groups, in the transposed S^T layout per 256-key chunk pair:
  S^T pair [s=2x128, q=512] (PE) -> exp (ACT) -> x 0/1 causal mask (DVE,
  diagonal chunks only) -> row-sums via ones-matmul + out^T AV
  accumulation (PE) -> out^T * (1/sums) (DVE approx-recip + mul) ->
  +bv, cast (ACT, per-partition bias in the out^T layout) -> DMA out^T.
The output is returned transposed ([H, TQ] per core); the host undoes it.
"""

import ml_dtypes
import numpy as np

import concourse.bacc as bacc
import concourse.mybir as mybir
import concourse.tile as tile
from concourse.bass_utils import run_bass_kernel_spmd

B, T, C, H = 4, 4096, 2048, 128
P = 128          # partitions / head dim / q tile
KB = 512         # free-dim tile (one f32 PSUM bank)
HB = 256         # per-core half of a column group
NQT = 16         # query tiles per core
TQ = NQT * P     # query rows per core
NCC = C // P     # contraction chunks (16)
NG = T // KB     # 512-wide column groups (8)
NM = 4           # attention q-groups per core (4 tiles each)
NMSK = 8 * NM    # 8 masked 128-key chunks per q-group

F16 = np.float16
F8 = ml_dtypes.float8_e4m3fn
_NC_CACHE = {}
REPLICA_GROUPS = [[0, 1], [2, 3], [4, 5], [6, 7]]


def _qtiles_for(half):
    # global query-tile ids, j-th tile of this core; kb counts [1,1,2,2,...,8,8]
    return [4 * (j // 2) + 2 * half + (j % 2) for j in range(NQT)]


def build_nc():
    dt = mybir.dt
    nc = bacc.Bacc("TRN2", target_bir_lowering=False, debug=False, num_devices=8)

    xP = nc.dram_tensor("xP", [NG, P, NCC, HB], dt.float16, kind="ExternalInput").ap()
    wk8 = nc.dram_tensor("wk8", [P, NCC, H], dt.float8e4, kind="ExternalInput").ap()
    wq8 = nc.dram_tensor("wq8", [P, NCC, H], dt.float8e4, kind="ExternalInput").ap()
    wv = nc.dram_tensor("wv", [P, NCC, H], dt.float16, kind="ExternalInput").ap()
    bk = nc.dram_tensor("bk", [P, 1], dt.float32, kind="ExternalInput").ap()
    bq = nc.dram_tensor("bq", [P, 1], dt.float32, kind="ExternalInput").ap()
    bv = nc.dram_tensor("bv", [P, 1], dt.float32, kind="ExternalInput").ap()
    consts = nc.dram_tensor("consts", [P, 2, P], dt.float16, kind="ExternalInput").ap()
    # causal-mask index tile: I[p, h2, q~] = q_global_local - 128*h2 - p;
    # keep key (2*pi + h2, p) for q~ of group m iff I >= 128*(2*pi - 8*m)
    imask = nc.dram_tensor("imask", [P, 2, KB], dt.float16, kind="ExternalInput").ap()
    outT = nc.dram_tensor("outT", [P, NM, KB], dt.float32, kind="ExternalOutput").ap()

    Exp = mybir.ActivationFunctionType.Exp
    Ident = mybir.ActivationFunctionType.Identity

    with tile.TileContext(nc) as tc:
        with (
            tc.tile_pool(name="wpool", bufs=1) as wpool,
            tc.tile_pool(name="cpool", bufs=1) as cpool,
            tc.tile_pool(name="persist", bufs=1) as persist,
            tc.tile_pool(name="mpool", bufs=1) as mpool,
            tc.tile_pool(name="xpool", bufs=3) as xpool,
            tc.tile_pool(name="vtpool", bufs=2) as vtpool,
            tc.tile_pool(name="kvpool", bufs=2) as kvpool,
            tc.tile_pool(name="dram", bufs=4, space="DRAM") as dram,
            tc.tile_pool(name="ppool", bufs=2, space="PSUM") as ppool,
            tc.tile_pool(name="spool", bufs=2, space="PSUM") as spool,
            tc.tile_pool(name="sumpool", bufs=1, space="PSUM") as sumpool,
            tc.tile_pool(name="otpool", bufs=1, space="PSUM") as otpool,
            tc.tile_pool(name="weipool", bufs=5) as weipool,
            tc.tile_pool(name="statpool", bufs=2) as statpool,
            tc.tile_pool(name="osbpool", bufs=2) as osbpool,
            tc.tile_pool(name="opool", bufs=2) as opool,
        ):
            # ---- constants (tiny ones first so slab 0 lands early) ----
            bk_t = cpool.tile([P, 1], dt.float32, tag="bk")
            bq_t = cpool.tile([P, 1], dt.float32, tag="bq")
            bv_t = cpool.tile([P, 1], dt.float32, tag="bv")
            idon = cpool.tile([P, 2, P], dt.float16, tag="idon")
            nc.sync.dma_start(bk_t[:], bk)
            nc.sync.dma_start(bq_t[:], bq)
            nc.sync.dma_start(bv_t[:], bv)
            nc.sync.dma_start(idon[:], consts)
            wk_t = wpool.tile([P, NCC, H], dt.float8e4, tag="wk")
            wq_t = wpool.tile([P, NCC, H], dt.float8e4, tag="wq")
            wv_t = wpool.tile([P, NCC, H], dt.float16, tag="wv")
            nc.sync.dma_start(wk_t[:], wk8)
            imask_t = mpool.tile([P, 2, KB], dt.float16, tag="imask")
            nc.sync.dma_start(imask_t[:], imask)

            kT = persist.tile([P, T], dt.float16, tag="kT")
            qT = persist.tile([P, TQ], dt.float16, tag="qT")
            vS = persist.tile([P, T // P, H], dt.float16, tag="vS")

            DR = mybir.MatmulPerfMode.DoubleRow
            fetched = {}

            def fetch(g):
                # DMA slab g and cast it to fp8 (issued ahead of use so the
                # cast sits early in the DVE queue)
                if g >= NG:
                    return
                xs = xpool.tile([P, NCC, HB], dt.float16, tag="xs", name=f"xs{g}")
                x8 = xpool.tile([P, NCC, HB], dt.float8e4, tag="x8", name=f"x8{g}")
                nc.sync.dma_start(xs[:], xP[g])
                nc.vector.tensor_copy(x8[:], xs[:])
                fetched[g] = (xs, x8)

            def project(g, kv2):
                xs, x8 = fetched.pop(g)
                fetch(g + 2)
                gg = g % 2
                # k^T of my half (fp8 DoubleRow, weights pre-scaled by 2^5)
                pk = ppool.tile([P, HB], dt.float32, tag="proj")
                for j in range(NCC // 2):
                    nc.tensor.matmul(
                        pk[:], lhsT=wk_t[:, 2 * j : 2 * j + 2, :],
                        rhs=x8[:, 2 * j : 2 * j + 2, :],
                        start=(j == 0), stop=(j == NCC // 2 - 1),
                        perf_mode=DR,
                    )
                nc.scalar.activation(
                    kv2[:, gg, 0, :], pk[:], Ident, bias=bk_t[:], scale=2.0 ** -5
                )
                # q^T for my two tiles (fp8 DoubleRow, weights pre-scaled by 2^10)
                pq = ppool.tile([P, HB], dt.float32, tag="proj")
                for j in range(NCC // 2):
                    nc.tensor.matmul(
                        pq[:], lhsT=wq_t[:, 2 * j : 2 * j + 2, :],
                        rhs=x8[:, 2 * j : 2 * j + 2, :],
                        start=(j == 0), stop=(j == NCC // 2 - 1),
                        perf_mode=DR,
                    )
                nc.scalar.activation(
                    qT[:, HB * g : HB * (g + 1)], pq[:], Ident, bias=bq_t[:],
                    scale=2.0 ** -10,
                )
                # v^T of my half (fp16)
                pv = ppool.tile([P, HB], dt.float32, tag="proj")
                for cc in range(NCC):
                    nc.tensor.matmul(
                        pv[:], lhsT=wv_t[:, cc, :], rhs=xs[:, cc, :],
                        start=(cc == 0), stop=(cc == NCC - 1),
                    )
                vt = vtpool.tile([P, HB], dt.float16, tag="vt")
                nc.scalar.copy(vt[:], pv[:])
                for s4 in range(2):
                    tp = ppool.tile([P, P], dt.float16, tag="proj")
                    nc.tensor.transpose(
                        tp[:], vt[:, P * s4 : P * (s4 + 1)], idon[:, 0, :]
                    )
                    nc.vector.tensor_copy(
                        kv2[:, gg, 1, P * s4 : P * (s4 + 1)], tp[:]
                    )

            def exchange(i):
                # AllGather (k^T | v^T) of groups 2i, 2i+1 with the pair partner.
                # cin staging on the scalar ring: the trigger lands right after
                # the k/q activations that produce kv, so it neither waits long
                # nor blocks the x-slab stream on the sync ring.
                cin = dram.tile([P, 2, 2, HB], dt.float16, tag="cin")
                cout = dram.tile([2, P, 2, 2, HB], dt.float16, tag="cout")
                nc.scalar.dma_start(cin[:], kvx[i % 2][:])
                nc.gpsimd.collective_compute(
                    "AllGather",
                    mybir.AluOpType.bypass,
                    replica_groups=REPLICA_GROUPS,
                    ins=[cin.opt()],
                    outs=[cout.opt()],
                )
                return cout

            def unpack(i, cout):
                # cout[r, :, gg] = (k^T | v^T) of group 2i+gg from core-half r
                for gg in range(2):
                    g = 2 * i + gg
                    nc.sync.dma_start(
                        kT[:, KB * g : KB * (g + 1)].rearrange(
                            "p (r h) -> p r h", r=2
                        ),
                        cout[:, :, gg, 0, :].rearrange("r p h -> p r h"),
                    )
                    nc.sync.dma_start(
                        vS[:, 4 * g : 4 * (g + 1), :].rearrange(
                            "p (r s) h -> p r s h", r=2
                        ),
                        cout[:, :, gg, 1, :].rearrange(
                            "r p (s h) -> p r s h", s=2
                        ),
                    )

            def attention(m):
                nch = 8 * m + 8     # 128-wide key chunks for this group
                npr = nch // 2
                sums = sumpool.tile([P, KB], dt.float32, tag="sums")
                otp = otpool.tile([P, KB], dt.float32, tag="otp")
                qg = qT[:, KB * m : KB * (m + 1)]
                wei_tiles = []

                def ones_av(p):
                    w = wei_tiles[p]
                    for h2 in range(2):
                        c = 2 * p + h2
                        nc.tensor.matmul(
                            sums[:], lhsT=idon[:, 1, :], rhs=w[:, h2, :],
                            start=(c == 0), stop=(c == nch - 1),
                        )
                        nc.tensor.matmul(
                            otp[:], lhsT=vS[:, c, :], rhs=w[:, h2, :],
                            start=(c == 0), stop=(c == nch - 1),
                        )

                for p in range(npr):
                    st = spool.tile([P, 2, KB], dt.float32, tag="st")
                    for h2 in range(2):
                        nc.tensor.matmul(
                            st[:, h2, :],
                            lhsT=kT[:, P * (2 * p + h2) : P * (2 * p + h2 + 1)],
                            rhs=qg, start=True, stop=True,
                        )
                    wei = weipool.tile([P, 2, KB], dt.float16, tag="wei")
                    nc.scalar.activation(wei[:], st[:], Exp)
                    if p >= npr - 4:
                        # causal mask: keep iff imask >= 128*(2*p - 8*m)
                        nc.vector.scalar_tensor_tensor(
                            wei[:], imask_t[:], float(128 * (2 * p - 8 * m)),
                            wei[:],
                            op0=mybir.AluOpType.is_ge,
                            op1=mybir.AluOpType.mult,
                        )
                    wei_tiles.append(wei)
                    if p > 0:
                        ones_av(p - 1)
                ones_av(npr - 1)
                rec = statpool.tile([P, KB], dt.float32, tag="rec")
                nc.vector.reciprocal_approx_fast(rec[:], sums[:])
                osb = osbpool.tile([P, KB], dt.float16, tag="osb")
                nc.vector.tensor_mul(osb[:], otp[:], rec[:])
                oT = opool.tile([P, KB], dt.float32, tag="oT")
                nc.scalar.activation(oT[:], osb[:], Ident, bias=bv_t[:])
                # scalar ring: fires right after the producing activation,
                # keeps the sync ring free for the x-slab stream
                nc.scalar.dma_start(outT[:, m, :], oT[:])

            # warm-up collective: absorbs CC-core boot + barrier skew while
            # the x/weight DMAs stream in
            cin_w = dram.tile([P, 2], dt.float16, tag="cinw")
            cout_w = dram.tile([2, P, 2], dt.float16, tag="coutw")
            nc.gpsimd.dma_start(cin_w[:], idon[:, 0, 0:2])
            nc.gpsimd.collective_compute(
                "AllGather",
                mybir.AluOpType.bypass,
                replica_groups=REPLICA_GROUPS,
                ins=[cin_w.opt()],
                outs=[cout_w.opt()],
            )

            # pipeline: projections feed attention groups as kT/vS fill in
            kvx = [
                kvpool.tile([P, 2, 2, HB], dt.float16, tag="kvx", name="kvx0"),
                kvpool.tile([P, 2, 2, HB], dt.float16, tag="kvx", name="kvx1"),
            ]
            fetch(0)
            nc.sync.dma_start(wq_t[:], wq8)
            nc.sync.dma_start(wv_t[:], wv)
            fetch(1)
            project(0, kvx[0])
            project(1, kvx[0])
            co0 = exchange(0)
            project(2, kvx[1])
            project(3, kvx[1])
            co1 = exchange(1)
            unpack(0, co0)
            attention(0)
            project(4, kvx[0])
            project(5, kvx[0])
            co2 = exchange(2)
            unpack(1, co1)
            attention(1)
            project(6, kvx[1])
            project(7, kvx[1])
            co3 = exchange(3)
            unpack(2, co2)
            unpack(3, co3)
            attention(2)
            attention(3)

    nc.compile()
    return nc


def _host_prep(x, Wk, bk, Wq, bq, Wv, bv):
    scale = float(C) ** -0.5

    def tile_w(w):
        return np.ascontiguousarray(
            w.reshape(NCC, P, H).transpose(1, 0, 2)
        )

    wk8c = tile_w((np.asarray(Wk, np.float32) * 2.0 ** 5).astype(F8))
    wq8c = tile_w((np.asarray(Wq, np.float32) * scale * 2.0 ** 10).astype(F8))
    wv16 = tile_w(np.asarray(Wv, np.float32).astype(F16))
    bk_c = np.asarray(bk, np.float32).reshape(P, 1)
    bq_c = (np.asarray(bq, np.float32) * scale).reshape(P, 1)
    bv_c = np.asarray(bv, np.float32).reshape(P, 1)
    consts = np.ascontiguousarray(
        np.stack([np.eye(P, dtype=F16), np.ones((P, P), F16)]).transpose(1, 0, 2)
    )

    # causal-mask index tile per half (m-independent; see kernel docstring)
    imask_by_half = []
    for half in (0, 1):
        I = np.empty((P, 2, KB), np.float32)
        for r in range(4):
            qloc = P * (2 * half + 4 * (r // 2) + (r % 2)) + np.arange(KB // 4)
            for h2 in range(2):
                I[:, h2, (KB // 4) * r : (KB // 4) * (r + 1)] = (
                    qloc[None, :] - P * h2 - np.arange(P)[:, None]
                )
        imask_by_half.append(I.astype(F16))

    in_maps = []
    for core in range(8):
        b_idx, half = core // 2, core % 2
        xTb = np.ascontiguousarray(np.asarray(x[b_idx], np.float32).T)
        xPc = np.empty((NG, P, NCC, HB), F16)
        for g in range(NG):
            grp = xTb[:, KB * g + HB * half : KB * g + HB * (half + 1)]
            xPc[g] = grp.reshape(NCC, P, HB).transpose(1, 0, 2).astype(F16)
        in_maps.append({
            "xP": xPc,
            "wk8": wk8c, "wq8": wq8c, "wv": wv16,
            "bk": bk_c, "bq": bq_c, "bv": bv_c,
            "consts": consts, "imask": imask_by_half[half],
        })
    return in_maps


def kernel(x, Wk, bk, Wq, bq, Wv, bv):
    if "nc" not in _NC_CACHE:
        _NC_CACHE["nc"] = build_nc()
    nc = _NC_CACHE["nc"]
    in_maps = _host_prep(x, Wk, bk, Wq, bq, Wv, bv)
    res = run_bass_kernel_spmd(nc, in_maps, list(range(8))).results
    out = np.empty((B, T, H), np.float32)
    for core in range(8):
        b_idx, half = core // 2, core % 2
        oT = res[core]["outT"]  # [P(H), NM, KB]
        qts = _qtiles_for(half)
        for j, qt in enumerate(qts):
            m, r = j // 4, j % 4
            out[b_idx, qt * P : (qt + 1) * P, :] = oT[:, m, r * P : (r + 1) * P].T
    return out


# revision 51
# speedup vs baseline: 1.1136x; 1.0403x over previous
"""Causal single-head attention (B=4, T=4096, C=2048, H=128) on 8 TRN2 cores.

Sharding: data-parallel over batch (2 cores per batch element). Within a
batch, core half h owns query tiles qt with qt mod 4 in {2h, 2h+1} — both
cores get an identical multiset of causal key-block counts, so one SPMD
program is balanced. Each core also projects k/v/q only for its own 2048
columns; (k^T | v^T) halves are exchanged with a pair-wise AllGather per
TWO 512-column groups (4 collectives total), halving both the x DMA
traffic and the k/v projection FLOPs.

Per-core device program (fp16 operands, f32 PSUM accumulation), pipelined
per column group g: project k^T / v^T / q^T of my 256 columns from slab
g, AllGather (k^T | v^T) of groups (2i, 2i+1) with the pair partner, and
interleave one attention q-group (4 query tiles, 512 q columns) per two
projection groups, in the transposed S^T layout per 256-key chunk pair:
  S^T pair [s=2x128, q=512] (PE) -> exp (ACT) -> x 0/1 causal mask (DVE,
  diagonal chunks only) -> row-sums via ones-matmul + out^T AV
  accumulation (PE) -> out^T * (1/sums) (DVE approx-recip + mul) ->
  +bv, cast (ACT, per-partition bias in the out^T layout) -> DMA out^T.
The output is returned transposed ([H, TQ] per core); the host undoes it.
"""

import ml_dtypes
import numpy as np

import concourse.bacc as bacc
import concourse.mybir as mybir
import concourse.tile as tile
from concourse.bass_utils import run_bass_kernel_spmd

B, T, C, H = 4, 4096, 2048, 128
P = 128          # partitions / head dim / q tile
KB = 512         # free-dim tile (one f32 PSUM bank)
HB = 256         # per-core half of a column group
NQT = 16         # query tiles per core
TQ = NQT * P     # query rows per core
NCC = C // P     # contraction chunks (16)
NG = T // KB     # 512-wide column groups (8)
NM = 4           # attention q-groups per core (4 tiles each)
NMSK = 8 * NM    # 8 masked 128-key chunks per q-group

F16 = np.float16
F8 = ml_dtypes.float8_e4m3fn
_NC_CACHE = {}
REPLICA_GROUPS = [[0, 1], [2, 3], [4, 5], [6, 7]]


def _qtiles_for(half):
    # global query-tile ids, j-th tile of this core; kb counts [1,1,2,2,...,8,8]
    return [4 * (j // 2) + 2 * half + (j % 2) for j in range(NQT)]


def build_nc():
    dt = mybir.dt
    nc = bacc.Bacc("TRN2", target_bir_lowering=False, debug=False, num_devices=8)

    xP = nc.dram_tensor("xP", [NG, P, NCC, HB], dt.float16, kind="ExternalInput").ap()
    wk8 = nc.dram_tensor("wk8", [P, NCC, H], dt.float8e4, kind="ExternalInput").ap()
    wq8 = nc.dram_tensor("wq8", [P, NCC, H], dt.float8e4, kind="ExternalInput").ap()
    wv = nc.dram_tensor("wv", [P, NCC, H], dt.float16, kind="ExternalInput").ap()
    bk = nc.dram_tensor("bk", [P, 1], dt.float32, kind="ExternalInput").ap()
    bq = nc.dram_tensor("bq", [P, 1], dt.float32, kind="ExternalInput").ap()
    bv = nc.dram_tensor("bv", [P, 1], dt.float32, kind="ExternalInput").ap()
    consts = nc.dram_tensor("consts", [P, 2, P], dt.float16, kind="ExternalInput").ap()
    # causal-mask index tile: I[p, h2, q~] = q_global_local - 128*h2 - p;
    # keep key (2*pi + h2, p) for q~ of group m iff I >= 128*(2*pi - 8*m)
    imask = nc.dram_tensor("imask", [P, 2, KB], dt.float16, kind="ExternalInput").ap()
    outT = nc.dram_tensor("outT", [P, NM, KB], dt.float32, kind="ExternalOutput").ap()

    Exp = mybir.ActivationFunctionType.Exp
    Ident = mybir.ActivationFunctionType.Identity

    with tile.TileContext(nc) as tc:
        with (
            tc.tile_pool(name="wpool", bufs=1) as wpool,
            tc.tile_pool(name="cpool", bufs=1) as cpool,
            tc.tile_pool(name="persist", bufs=1) as persist,
            tc.tile_pool(name="mpool", bufs=1) as mpool,
            tc.tile_pool(name="xpool", bufs=3) as xpool,
            tc.tile_pool(name="vtpool", bufs=2) as vtpool,
            tc.tile_pool(name="kvpool", bufs=2) as kvpool,
            tc.tile_pool(name="dram", bufs=4, space="DRAM") as dram,
            tc.tile_pool(name="ppool", bufs=2, space="PSUM") as ppool,
            tc.tile_pool(name="spool", bufs=2, space="PSUM") as spool,
            tc.tile_pool(name="sumpool", bufs=1, space="PSUM") as sumpool,
            tc.tile_pool(name="otpool", bufs=1, space="PSUM") as otpool,
            tc.tile_pool(name="weipool", bufs=5) as weipool,
            tc.tile_pool(name="statpool", bufs=2) as statpool,
            tc.tile_pool(name="osbpool", bufs=2) as osbpool,
            tc.tile_pool(name="opool", bufs=2) as opool,
        ):
            # ---- constants (tiny ones first so slab 0 lands early) ----
            bk_t = cpool.tile([P, 1], dt.float32, tag="bk")
            bq_t = cpool.tile([P, 1], dt.float32, tag="bq")
            bv_t = cpool.tile([P, 1], dt.float32, tag="bv")
            idon = cpool.tile([P, 2, P], dt.float16, tag="idon")
            nc.sync.dma_start(bk_t[:], bk)
            nc.sync.dma_start(bq_t[:], bq)
            nc.sync.dma_start(bv_t[:], bv)
            nc.sync.dma_start(idon[:], consts)
            wk_t = wpool.tile([P, NCC, H], dt.float8e4, tag="wk")
            wq_t = wpool.tile([P, NCC, H], dt.float8e4, tag="wq")
            wv_t = wpool.tile([P, NCC, H], dt.float16, tag="wv")
            nc.sync.dma_start(wk_t[:], wk8)
            imask_t = mpool.tile([P, 2, KB], dt.float16, tag="imask")
            nc.sync.dma_start(imask_t[:], imask)

            kT = persist.tile([P, T], dt.float16, tag="kT")
            qT = persist.tile([P, TQ], dt.float16, tag="qT")
            vS = persist.tile([P, T // P, H], dt.float16, tag="vS")

            DR = mybir.MatmulPerfMode.DoubleRow
            fetched = {}

            def fetch(g):
                # DMA slab g and cast it to fp8 (issued ahead of use so the
                # cast sits early in the DVE queue)
                if g >= NG:
                    return
                xs = xpool.tile([P, NCC, HB], dt.float16, tag="xs", name=f"xs{g}")
                x8 = xpool.tile([P, NCC, HB], dt.float8e4, tag="x8", name=f"x8{g}")
                nc.sync.dma_start(xs[:], xP[g])
                nc.vector.tensor_copy(x8[:], xs[:])
                fetched[g] = (xs, x8)

            def project(g, kv2):
                xs, x8 = fetched.pop(g)
                fetch(g + 2)
                gg = g % 2
                # k^T of my half (fp8 DoubleRow, weights pre-scaled by 2^5)
                pk = ppool.tile([P, HB], dt.float32, tag="proj")
                for j in range(NCC // 2):
                    nc.tensor.matmul(
                        pk[:], lhsT=wk_t[:, 2 * j : 2 * j + 2, :],
                        rhs=x8[:, 2 * j : 2 * j + 2, :],
                        start=(j == 0), stop=(j == NCC // 2 - 1),
                        perf_mode=DR,
                    )
                nc.scalar.activation(
                    kv2[:, gg, 0, :], pk[:], Ident, bias=bk_t[:], scale=2.0 ** -5
                )
                # q^T for my two tiles (fp8 DoubleRow, weights pre-scaled by 2^10)
                pq = ppool.tile([P, HB], dt.float32, tag="proj")
                for j in range(NCC // 2):
                    nc.tensor.matmul(
                        pq[:], lhsT=wq_t[:, 2 * j : 2 * j + 2, :],
                        rhs=x8[:, 2 * j : 2 * j + 2, :],
                        start=(j == 0), stop=(j == NCC // 2 - 1),
                        perf_mode=DR,
                    )
                nc.scalar.activation(
                    qT[:, HB * g : HB * (g + 1)], pq[:], Ident, bias=bq_t[:],
                    scale=2.0 ** -10,
                )
                # v^T of my half (fp16)
                pv = ppool.tile([P, HB], dt.float32, tag="proj")
                for cc in range(NCC):
                    nc.tensor.matmul(
                        pv[:], lhsT=wv_t[:, cc, :], rhs=xs[:, cc, :],
                        start=(cc == 0), stop=(cc == NCC - 1),
                    )
                vt = vtpool.tile([P, HB], dt.float16, tag="vt")
                nc.scalar.copy(vt[:], pv[:])
                for s4 in range(2):
                    tp = ppool.tile([P, P], dt.float16, tag="proj")
                    nc.tensor.transpose(
                        tp[:], vt[:, P * s4 : P * (s4 + 1)], idon[:, 0, :]
                    )
                    nc.vector.tensor_copy(
                        kv2[:, gg, 1, P * s4 : P * (s4 + 1)], tp[:]
                    )

            def exchange(i):
                # AllGather (k^T | v^T) of groups 2i, 2i+1 with the pair partner.
                # cin staging on the scalar ring: the trigger lands right after
                # the k/q activations that produce kv, so it neither waits long
                # nor blocks the x-slab stream on the sync ring.
                cin = dram.tile([P, 2, 2, HB], dt.float16, tag="cin")
                cout = dram.tile([2, P, 2, 2, HB], dt.float16, tag="cout")
                nc.scalar.dma_start(cin[:], kvx[i % 2][:])
                nc.gpsimd.collective_compute(
                    "AllGather",
                    mybir.AluOpType.bypass,
                    replica_groups=REPLICA_GROUPS,
                    ins=[cin.opt()],
                    outs=[cout.opt()],
                )
                return cout

            def unpack(i, cout):
                # cout[r, :, gg] = (k^T | v^T) of group 2i+gg from core-half r
                for gg in range(2):
                    g = 2 * i + gg
                    nc.gpsimd.dma_start(
                        kT[:, KB * g : KB * (g + 1)].rearrange(
                            "p (r h) -> p r h", r=2
                        ),
                        cout[:, :, gg, 0, :].rearrange("r p h -> p r h"),
                    )
                    nc.gpsimd.dma_start(
                        vS[:, 4 * g : 4 * (g + 1), :].rearrange(
                            "p (r s) h -> p r s h", r=2
                        ),
                        cout[:, :, gg, 1, :].rearrange(
                            "r p (s h) -> p r s h", s=2
                        ),
                    )

            def attention(m):
                nch = 8 * m + 8     # 128-wide key chunks for this group
                npr = nch // 2
                sums = sumpool.tile([P, KB], dt.float32, tag="sums")
                otp = otpool.tile([P, KB], dt.float32, tag="otp")
                qg = qT[:, KB * m : KB * (m + 1)]
                wei_tiles = []

                def ones_av(p):
                    w = wei_tiles[p]
                    for h2 in range(2):
                        c = 2 * p + h2
                        nc.tensor.matmul(
                            sums[:], lhsT=idon[:, 1, :], rhs=w[:, h2, :],
                            start=(c == 0), stop=(c == nch - 1),
                        )
                        nc.tensor.matmul(
                            otp[:], lhsT=vS[:, c, :], rhs=w[:, h2, :],
                            start=(c == 0), stop=(c == nch - 1),
                        )

                for p in range(npr):
                    st = spool.tile([P, 2, KB], dt.float32, tag="st")
                    for h2 in range(2):
                        nc.tensor.matmul(
                            st[:, h2, :],
                            lhsT=kT[:, P * (2 * p + h2) : P * (2 * p + h2 + 1)],
                            rhs=qg, start=True, stop=True,
                        )
                    wei = weipool.tile([P, 2, KB], dt.float16, tag="wei")
                    nc.scalar.activation(wei[:], st[:], Exp)
                    if p >= npr - 4:
                        # causal mask: keep iff imask >= 128*(2*p - 8*m)
                        nc.vector.scalar_tensor_tensor(
                            wei[:], imask_t[:], float(128 * (2 * p - 8 * m)),
                            wei[:],
                            op0=mybir.AluOpType.is_ge,
                            op1=mybir.AluOpType.mult,
                        )
                    wei_tiles.append(wei)
                    if p > 0:
                        ones_av(p - 1)
                ones_av(npr - 1)
                rec = statpool.tile([P, KB], dt.float32, tag="rec")
                nc.vector.reciprocal_approx_fast(rec[:], sums[:])
                osb = osbpool.tile([P, KB], dt.float16, tag="osb")
                nc.vector.tensor_mul(osb[:], otp[:], rec[:])
                oT = opool.tile([P, KB], dt.float32, tag="oT")
                nc.scalar.activation(oT[:], osb[:], Ident, bias=bv_t[:])
                # scalar ring: fires right after the producing activation,
                # keeps the sync ring free for the x-slab stream
                nc.scalar.dma_start(outT[:, m, :], oT[:])

            # warm-up collective: absorbs CC-core boot + barrier skew while
            # the x/weight DMAs stream in
            cin_w = dram.tile([P, 2], dt.float16, tag="cinw")
            cout_w = dram.tile([2, P, 2], dt.float16, tag="coutw")
            nc.gpsimd.dma_start(cin_w[:], idon[:, 0, 0:2])
            nc.gpsimd.collective_compute(
                "AllGather",
                mybir.AluOpType.bypass,
                replica_groups=REPLICA_GROUPS,
                ins=[cin_w.opt()],
                outs=[cout_w.opt()],
            )

            # pipeline: projections feed attention groups as kT/vS fill in
            kvx = [
                kvpool.tile([P, 2, 2, HB], dt.float16, tag="kvx", name="kvx0"),
                kvpool.tile([P, 2, 2, HB], dt.float16, tag="kvx", name="kvx1"),
            ]
            fetch(0)
            nc.sync.dma_start(wq_t[:], wq8)
            nc.sync.dma_start(wv_t[:], wv)
            fetch(1)
            project(0, kvx[0])
            project(1, kvx[0])
            co0 = exchange(0)
            project(2, kvx[1])
            project(3, kvx[1])
            co1 = exchange(1)
            unpack(0, co0)
            attention(0)
            project(4, kvx[0])
            project(5, kvx[0])
            co2 = exchange(2)
            unpack(1, co1)
            attention(1)
            project(6, kvx[1])
            project(7, kvx[1])
            co3 = exchange(3)
            unpack(2, co2)
            unpack(3, co3)
            attention(2)
            attention(3)

    nc.compile()
    return nc


def _host_prep(x, Wk, bk, Wq, bq, Wv, bv):
    scale = float(C) ** -0.5

    def tile_w(w):
        return np.ascontiguousarray(
            w.reshape(NCC, P, H).transpose(1, 0, 2)
        )

    wk8c = tile_w((np.asarray(Wk, np.float32) * 2.0 ** 5).astype(F8))
    wq8c = tile_w((np.asarray(Wq, np.float32) * scale * 2.0 ** 10).astype(F8))
    wv16 = tile_w(np.asarray(Wv, np.float32).astype(F16))
    bk_c = np.asarray(bk, np.float32).reshape(P, 1)
    bq_c = (np.asarray(bq, np.float32) * scale).reshape(P, 1)
    bv_c = np.asarray(bv, np.float32).reshape(P, 1)
    consts = np.ascontiguousarray(
        np.stack([np.eye(P, dtype=F16), np.ones((P, P), F16)]).transpose(1, 0, 2)
    )

    # causal-mask index tile per half (m-independent; see kernel docstring)
    imask_by_half = []
    for half in (0, 1):
        I = np.empty((P, 2, KB), np.float32)
        for r in range(4):
            qloc = P * (2 * half + 4 * (r // 2) + (r % 2)) + np.arange(KB // 4)
            for h2 in range(2):
                I[:, h2, (KB // 4) * r : (KB // 4) * (r + 1)] = (
                    qloc[None, :] - P * h2 - np.arange(P)[:, None]
                )
        imask_by_half.append(I.astype(F16))

    in_maps = []
    for core in range(8):
        b_idx, half = core // 2, core % 2
        xTb = np.ascontiguousarray(np.asarray(x[b_idx], np.float32).T)
        xPc = np.empty((NG, P, NCC, HB), F16)
        for g in range(NG):
            grp = xTb[:, KB * g + HB * half : KB * g + HB * (half + 1)]
            xPc[g] = grp.reshape(NCC, P, HB).transpose(1, 0, 2).astype(F16)
        in_maps.append({
            "xP": xPc,
            "wk8": wk8c, "wq8": wq8c, "wv": wv16,
            "bk": bk_c, "bq": bq_c, "bv": bv_c,
            "consts": consts, "imask": imask_by_half[half],
        })
    return in_maps


def kernel(x, Wk, bk, Wq, bq, Wv, bv):
    if "nc" not in _NC_CACHE:
        _NC_CACHE["nc"] = build_nc()
    nc = _NC_CACHE["nc"]
    in_maps = _host_prep(x, Wk, bk, Wq, bq, Wv, bv)
    res = run_bass_kernel_spmd(nc, in_maps, list(range(8))).results
    out = np.empty((B, T, H), np.float32)
    for core in range(8):
        b_idx, half = core // 2, core % 2
        oT = res[core]["outT"]  # [P(H), NM, KB]
        qts = _qtiles_for(half)
        for j, qt in enumerate(qts):
            m, r = j // 4, j % 4
            out[b_idx, qt * P : (qt + 1) * P, :] = oT[:, m, r * P : (r + 1) * P].T
    return out
